# revision 35
# baseline (speedup 1.0000x reference)
"""Distributed GQA attention prefill kernel for one TRN2 chip (8 NeuronCores).

Sharding: tensor-parallel over heads (4-way) x data-parallel over batch (2-way).
Core c handles batch b=c//4, TP rank r=c%4 (8 q-heads, 2 kv-heads each).

v3: bf16 end-to-end (fp32 PSUM accumulation), host-pretiled contiguous DMA
layouts, all weights resident in SBUF (loaded once), V kept in SBUF, RoPE
fused into the projection pass with deferred pswap matmuls (no PE stalls),
software-pipelined attention inner loop (score k+1 issued before PV k so the
tensor engine never waits on the exp), and the ReduceScatter split into one
chunk per q-tile so it overlaps with subsequent compute.
"""

import os
import sys
import numpy as np

B, S, D = 2, 2048, 4096
H, KV, HD = 32, 8, 128
TP = 4
QH = H // TP          # 8 q heads per core
G = KV // TP          # 2 kv heads per core
P = 128
QT = 512              # q-tile for attention/projection (free dim)
NQT = S // QT         # 4
JT = 256              # projection j-tile (free dim)
NJT = S // JT         # 8
NDK = D // P          # 32 contraction chunks of 128
SCALE = float(HD) ** -0.5

LAST_EXEC_NS = None
LAST_TRACE_DIR = None


def _build():
    sys.path.insert(0, "/opt/trn_rl_repo")
    import concourse.bass as bass
    from concourse import bacc
    import concourse.mybir as mybir
    import concourse.tile as tile
    from contextlib import ExitStack

    F32 = mybir.dt.float32
    F32R = mybir.dt.float32r
    BF = mybir.dt.bfloat16
    Exp = mybir.ActivationFunctionType.Exp
    Copy = mybir.ActivationFunctionType.Copy
    MUL = mybir.AluOpType.mult
    ADD = mybir.AluOpType.add

    nc = bacc.Bacc(None, target_bir_lowering=False)
    xt_e = nc.dram_tensor("xt", [NJT, P, NDK, JT], BF, kind="ExternalInput")
    wq_e = nc.dram_tensor("wq", [QH, P, NDK, HD], BF, kind="ExternalInput")
    wk_e = nc.dram_tensor("wk", [P, NDK, G * HD], BF, kind="ExternalInput")
    wv_e = nc.dram_tensor("wv", [P, NDK, G * HD], BF, kind="ExternalInput")
    wo_e = nc.dram_tensor("wo", [P, QH, D], BF, kind="ExternalInput")
    cost_e = nc.dram_tensor("cost", [P, S], BF, kind="ExternalInput")
    sint_e = nc.dram_tensor("sint", [P, S], BF, kind="ExternalInput")
    mbig_e = nc.dram_tensor("mbig", [P, 1024], BF, kind="ExternalInput")
    onec_e = nc.dram_tensor("onec", [P, 1], BF, kind="ExternalInput")
    oner_e = nc.dram_tensor("oner", [1, P], F32, kind="ExternalInput")
    pswap_e = nc.dram_tensor("pswap", [P, P], BF, kind="ExternalInput")
    NO_CC = os.environ.get("KERNEL_NO_CC", "0") == "1"
    out_shape = [S, D] if NO_CC else [NQT, P, D]
    out_e = nc.dram_tensor("out", out_shape, BF, kind="ExternalOutput")

    with ExitStack() as top:
        top.enter_context(nc.allow_low_precision(reason="bf16 attention"))
        tc = top.enter_context(tile.TileContext(nc))
        const = top.enter_context(tc.tile_pool(name="const", bufs=1))
        mbig = const.tile([P, 1024], BF)
        onec = const.tile([P, 1], BF)
        oner = const.tile([1, P], F32)
        cosT = const.tile([P, S], BF)
        sinT = const.tile([P, S], BF)
        pswap = const.tile([P, P], BF)

        pers = top.enter_context(tc.tile_pool(name="pers", bufs=1))
        qT = pers.tile([P, QH, S], BF, name="qT")
        kT = pers.tile([P, G, S], BF, name="kT")
        vsb = pers.tile([P, S // P, G * HD], BF, name="vsb")

        dram = top.enter_context(tc.tile_pool(name="dram", bufs=1,
                                              space="DRAM"))
        partall = dram.tile([S, D], BF, name="partall")
        ccout = None if NO_CC else dram.tile([NQT, P, D], BF, name="ccout")

        # ---------------- phase A: QKV projections + RoPE ----------------
        with tc.tile_pool(name="wqkv", bufs=1) as w_pool, \
             tc.tile_pool(name="xtp", bufs=2) as xt_pool, \
             tc.tile_pool(name="rtmp", bufs=3) as rtmp_pool, \
             tc.tile_pool(name="psA", bufs=4, space="PSUM") as psA:
            # x-tile j=0 first so the first projection chain starts early
            xt_next = xt_pool.tile([P, NDK, JT], BF, name="xt_t")
            nc.sync.dma_start(xt_next[:], xt_e[0])
            wq_sb = []
            for h in range(QH):
                w = w_pool.tile([P, NDK, HD], BF, name=f"wq{h}")
                nc.sync.dma_start(w[:], wq_e[h])
                wq_sb.append(w)
            wk_sb = w_pool.tile([P, NDK, G * HD], BF, name="wk_sb")
            nc.sync.dma_start(wk_sb[:], wk_e[:])
            wv_sb = w_pool.tile([P, NDK, G * HD], BF, name="wv_sb")
            nc.sync.dma_start(wv_sb[:], wv_e[:])
            # consts are not needed until the first RoPE / attention, so
            # they queue behind the latency-critical xt/weight loads
            nc.sync.dma_start(cosT[:], cost_e[:])
            nc.sync.dma_start(sinT[:], sint_e[:])
            nc.sync.dma_start(pswap[:], pswap_e[:])
            nc.sync.dma_start(onec[:], onec_e[:])
            nc.sync.dma_start(oner[:], oner_e[:])
            nc.sync.dma_start(mbig[:], mbig_e[:])

            for j in range(NJT):
                sl = slice(j * JT, (j + 1) * JT)
                xt_t = xt_next
                if j + 1 < NJT:
                    xt_next = xt_pool.tile([P, NDK, JT], BF, name="xt_t")
                    nc.sync.dma_start(xt_next[:], xt_e[j + 1])
                ropes = []
                for h in range(QH):
                    ps = psA.tile([P, JT], F32, tag="qk")
                    for dk in range(NDK):
                        nc.tensor.matmul(
                            ps[:], wq_sb[h][:, dk, :], xt_t[:, dk, :],
                            start=(dk == 0), stop=(dk == NDK - 1))
                    tmp = rtmp_pool.tile([P, JT], BF, tag="rt", bufs=12)
                    nc.scalar.activation(tmp[:], ps[:], Copy)
                    ropes.append((tmp, qT[:, h, sl]))
                for g in range(G):
                    ps = psA.tile([P, JT], F32, tag="qk")
                    for dk in range(NDK):
                        nc.tensor.matmul(
                            ps[:], wk_sb[:, dk, g * HD:(g + 1) * HD],
                            xt_t[:, dk, :],
                            start=(dk == 0), stop=(dk == NDK - 1))
                    tmp = rtmp_pool.tile([P, JT], BF, tag="rt", bufs=12)
                    nc.scalar.activation(tmp[:], ps[:], Copy)
                    ropes.append((tmp, kT[:, g, sl]))
                for sub in range(2):
                    psv = psA.tile([P, G * HD], F32, tag="v", bufs=2)
                    for dk in range(NDK):
                        nc.tensor.matmul(
                            psv[:], xt_t[:, dk, sub * P:(sub + 1) * P],
                            wv_sb[:, dk, :],
                            start=(dk == 0), stop=(dk == NDK - 1))
                    nc.scalar.activation(vsb[:, j * 2 + sub, :], psv[:], Copy)
                # deferred RoPE: pswap matmuls run back-to-back on PE, the
                # vector work overlaps the next j-tile's projection chains
                for tmp, dst in ropes:
                    ps2 = psA.tile([P, JT], F32, tag="rope", bufs=2)
                    nc.tensor.matmul(ps2[:], pswap[:], tmp[:],
                                     start=True, stop=True)
                    d1 = rtmp_pool.tile([P, JT], BF, tag="d1", bufs=2)
                    nc.vector.tensor_tensor(d1[:], tmp[:], cosT[:, sl], MUL)
                    d2 = rtmp_pool.tile([P, JT], BF, tag="d2", bufs=2)
                    nc.vector.tensor_tensor(d2[:], ps2[:], sinT[:, sl], MUL)
                    nc.vector.tensor_tensor(dst, d1[:], d2[:], ADD)

        # ---------------- phase B: attention + output projection ----------------
        with tc.tile_pool(name="wop", bufs=1) as wo_pool, \
             tc.tile_pool(name="attn", bufs=2) as attn_pool, \
             tc.tile_pool(name="probs", bufs=3) as probs_pool, \
             tc.tile_pool(name="osbp", bufs=8) as osb_pool, \
             tc.tile_pool(name="rp", bufs=2) as rp_pool, \
             tc.tile_pool(name="psB", bufs=1, space="PSUM") as psB:
            wo_sb = wo_pool.tile([P, QH, D], BF, name="wo_sb")
            nc.sync.dma_start(wo_sb[:], wo_e[:])

            for t in range(NQT):
                q0 = t * QT
                sl = slice(q0, q0 + QT)
                nk = 4 * (t + 1)
                attn_t = attn_pool.tile([P, QH, QT], BF, tag="attn")

                # software-pipelined attention: score (h,ki) is issued one
                # step ahead of pv/sum of the previous pair, so the PE never
                # waits on the scalar-engine exp
                pairs = [(h, ki) for h in range(QH) for ki in range(nk)]
                state = {}

                def score(h, ki):
                    # diagonal blocks only cover q >= k: narrow the free dim
                    g = h // 4
                    k0 = ki * P
                    off = k0 - q0
                    o = max(off, 0)      # q-offset within the tile
                    w = QT - o
                    ps_s = psB.tile([P, QT], F32, tag="s", bufs=3)
                    nc.tensor.matmul(
                        ps_s[:, :w], kT[:, g, k0:k0 + P],
                        qT[:, h, q0 + o:q0 + QT],
                        start=True, stop=True)
                    pr = probs_pool.tile([P, QT], BF, tag="pr", bufs=4)
                    nc.scalar.activation(pr[:, :w], ps_s[:, :w], Exp,
                                         scale=SCALE)
                    if off >= 0:
                        nc.vector.tensor_tensor(
                            pr[:, :w], pr[:, :w],
                            mbig[:, 512:1024 - off], MUL)
                    return pr, o, w

                def post(h, ki, pr, o, w):
                    g = h // 4
                    if ki == 0:
                        state[h] = (
                            psB.tile([P, QT], F32, tag="pv", bufs=2,
                                     name="pv_ps"),
                            psB.tile([1, QT], F32, tag="sm", bufs=1,
                                     name="sm_ps"))
                    pv, sm = state[h]
                    nc.tensor.matmul(
                        pv[:, o:], vsb[:, ki, g * HD:(g + 1) * HD],
                        pr[:, :w],
                        start=(ki == 0), stop=(ki == nk - 1))
                    nc.tensor.matmul(sm[:, o:], onec[:], pr[:, :w],
                                     start=(ki == 0), stop=(ki == nk - 1))
                    if ki == nk - 1:
                        recip = rp_pool.tile([1, QT], F32, tag="recip")
                        nc.vector.reciprocal_approx_fast(recip[:], sm[:])
                        state[h] = (pv, recip)

                def finish(h):
                    pv, recip = state.pop(h)
                    ps_b = psB.tile([P, QT], F32, tag="s", bufs=3)
                    nc.tensor.matmul(ps_b[:], oner[:], recip[:],
                                     start=True, stop=True)
                    dst = attn_t[:, h, :]
                    nc.scalar.activation(dst, pv[:], Copy)
                    nc.vector.tensor_tensor(dst, dst, ps_b[:], MUL)

                pend = []
                done_h = -1
                for h, ki in pairs:
                    pr, o, w = score(h, ki)
                    if len(pend) == 2:
                        pp = pend.pop(0)
                        post(*pp)
                        if pp[1] == nk - 1:
                            done_h = pp[0]
                    if ki == 3 and done_h >= 0 and done_h == h - 1:
                        finish(done_h)
                        done_h = -1
                    pend.append((h, ki, pr, o, w))
                for pp in pend:
                    post(*pp)
                finish(QH - 1)

                # output projection for this q-tile. The ReduceScatter for
                # tiles 0-2 is emitted whole right after the projection so
                # its ring DMA overlaps the (write-free) attention of the
                # next tile; the last tile is split in half so most of it
                # overlaps the tail of the projection.
                groups = [[0, 1, 2, 3], [4, 5, 6, 7]]
                for si in range(4):
                    r0 = q0 + si * P
                    # full-width staging tile: one partall write per strip
                    # with 8KB-contiguous lines instead of eight 1KB-line DMAs
                    osb = osb_pool.tile([P, D], BF, tag="ob", bufs=3)
                    for n in range(8):
                        n0 = n * QT
                        ps_o = psB.tile([P, QT], F32, tag="proj", bufs=2)
                        for hh in range(QH):
                            nc.tensor.matmul(
                                ps_o[:], attn_t[:, hh, si * P:(si + 1) * P],
                                wo_sb[:, hh, n0:n0 + QT],
                                start=(hh == 0), stop=(hh == QH - 1))
                        nc.scalar.activation(osb[:, n0:n0 + QT], ps_o[:],
                                             Copy)
                    nc.sync.dma_start(partall[r0:r0 + P, :], osb[:])
                    if not NO_CC and t == NQT - 1:
                        s32 = P // TP
                        nc.gpsimd.collective_compute(
                            "ReduceScatter", ADD, replica_groups=groups,
                            ins=[partall[r0:r0 + P, :].opt()],
                            outs=[ccout[t, si * s32:(si + 1) * s32, :].opt()])
                        nc.sync.dma_start(
                            out_e[t, si * s32:(si + 1) * s32, :],
                            ccout[t, si * s32:(si + 1) * s32, :])
                if NO_CC:
                    nc.sync.dma_start(
                        out_e[q0:q0 + QT, :], partall[q0:q0 + QT, :])
                elif t < NQT - 1:
                    nc.gpsimd.collective_compute(
                        "ReduceScatter", ADD, replica_groups=groups,
                        ins=[partall[q0:q0 + QT, :].opt()],
                        outs=[ccout[t].opt()])
                    nc.sync.dma_start(out_e[t], ccout[t])

    nc.compile()
    return nc


def _prep_in_maps(x, wq, wk, wv, wo, cos, sin):
    import ml_dtypes
    BF = ml_dtypes.bfloat16

    cosT = np.empty((HD, S), np.float32)
    sinT = np.empty((HD, S), np.float32)
    cosT[0::2] = cos.T
    cosT[1::2] = cos.T
    sinT[0::2] = -sin.T
    sinT[1::2] = sin.T
    cosT = cosT.astype(BF)
    sinT = sinT.astype(BF)
    mbig = (np.arange(1024)[None, :] >= (np.arange(P)[:, None] + 512)
            ).astype(BF)
    onec = np.ones((P, 1), BF)
    oner = np.ones((1, P), np.float32)
    pswap = np.zeros((P, P), np.float32)
    idx = np.arange(P)
    pswap[idx, idx ^ 1] = 1.0
    pswap = pswap.astype(BF)

    # x[b] pretiled: xt[j, p, dk*JT+s'] = x[b, j*JT+s', dk*P+p]
    xts = []
    for b in range(B):
        xr = x[b].reshape(NJT, JT, NDK, P)
        xts.append(np.ascontiguousarray(
            xr.transpose(0, 3, 2, 1)).astype(BF))

    in_maps = []
    for c in range(8):
        b, rk = c // TP, c % TP
        # wq head-major: wqp[h, p, dk*HD+m] = wq[dk*P+p, (rk*QH+h)*HD+m]
        wq_s = wq[:, rk * QH * HD:(rk + 1) * QH * HD]
        wqp = np.ascontiguousarray(
            wq_s.reshape(NDK, P, QH, HD).transpose(2, 1, 0, 3)).astype(BF)
        wk_s = wk[:, rk * G * HD:(rk + 1) * G * HD]
        wv_s = wv[:, rk * G * HD:(rk + 1) * G * HD]
        wo_s = wo[rk * QH * HD:(rk + 1) * QH * HD, :]
        in_maps.append({
            "xt": xts[b],
            "wq": wqp,
            "wk": np.ascontiguousarray(
                wk_s.reshape(NDK, P, G * HD).transpose(1, 0, 2)).astype(BF),
            "wv": np.ascontiguousarray(
                wv_s.reshape(NDK, P, G * HD).transpose(1, 0, 2)).astype(BF),
            "wo": np.ascontiguousarray(
                wo_s.reshape(QH, P, D).transpose(1, 0, 2)).astype(BF),
            "cost": cosT, "sint": sinT, "mbig": mbig,
            "onec": onec, "oner": oner, "pswap": pswap,
        })
    return in_maps


def _ensure_profile_hook():
    """Register the axon NTFF profile hook if the antenv plumbing is absent."""
    try:
        import antenv.axon_hooks  # noqa: F401
        return
    except ImportError:
        pass
    import types
    mod = types.ModuleType("antenv.axon_hooks")
    mod._HOOK = None

    def set_axon_ntff_profile_hook(hook):
        mod._HOOK = hook

    def get_axon_ntff_profile_hook():
        return mod._HOOK

    mod.set_axon_ntff_profile_hook = set_axon_ntff_profile_hook
    mod.get_axon_ntff_profile_hook = get_axon_ntff_profile_hook
    sys.modules["antenv.axon_hooks"] = mod
    try:
        from trn_agent_boot.trn_boot import _ntff_profile_via_ctypes
        hook = _ntff_profile_via_ctypes("/opt/axon/libaxon_pjrt.so")
        if hook is not None:
            mod._HOOK = hook
    except Exception:
        pass


def kernel(x, wq, wk, wv, wo, cos, sin, mask=None, positions=None, **_):
    global LAST_EXEC_NS, LAST_TRACE_DIR
    x = np.asarray(x, np.float32)
    wq = np.asarray(wq, np.float32)
    wk = np.asarray(wk, np.float32)
    wv = np.asarray(wv, np.float32)
    wo = np.asarray(wo, np.float32)
    cos = np.asarray(cos, np.float32)
    sin = np.asarray(sin, np.float32)

    sys.path.insert(0, "/opt/trn_rl_repo")
    from concourse.bass_utils import run_bass_kernel_spmd

    nc = _build()
    in_maps = _prep_in_maps(x, wq, wk, wv, wo, cos, sin)
    want_trace = os.environ.get("BASS_TRACE", "1") != "0"
    res = None
    if want_trace:
        try:
            _ensure_profile_hook()
            res = run_bass_kernel_spmd(nc, in_maps, list(range(8)), trace=True)
        except Exception:
            res = None
    if res is None:
        res = run_bass_kernel_spmd(nc, in_maps, list(range(8)), trace=False)
    LAST_EXEC_NS = res.exec_time_ns
    if LAST_EXEC_NS is None and os.environ.get("BASS_WALLTIME", "1") == "1":
        import time as _time
        t0 = _time.perf_counter()
        res = run_bass_kernel_spmd(nc, in_maps, list(range(8)), trace=False)
        LAST_EXEC_NS = int((_time.perf_counter() - t0) * 1e9)
    try:
        LAST_TRACE_DIR = getattr(res, "profile_json", None)
    except Exception:
        LAST_TRACE_DIR = None

    out = np.empty((B, S, D), np.float32)
    if os.environ.get("KERNEL_NO_CC", "0") == "1":
        for b in range(B):
            out[b] = sum(
                res.results[b * TP + rk]["out"].astype(np.float32)
                for rk in range(TP))
    else:
        for c in range(8):
            b, rk = c // TP, c % TP
            o = res.results[c]["out"].astype(np.float32)
            for t in range(NQT - 1):
                r0 = t * QT + rk * P
                out[b, r0:r0 + P, :] = o[t]
            # last tile was reduce-scattered per 128-row strip
            t = NQT - 1
            s32 = P // TP
            for si in range(4):
                r0 = t * QT + si * P + rk * s32
                out[b, r0:r0 + s32, :] = o[t, si * s32:(si + 1) * s32]
    return out


# revision 42
# speedup vs baseline: 1.0411x; 1.0411x over previous
"""Distributed GQA attention prefill kernel for one TRN2 chip (8 NeuronCores).

Sharding: tensor-parallel over heads (4-way) x data-parallel over batch (2-way).
Core c handles batch b=c//4, TP rank r=c%4 (8 q-heads, 2 kv-heads each).

v3: bf16 end-to-end (fp32 PSUM accumulation), host-pretiled contiguous DMA
layouts, all weights resident in SBUF (loaded once), V kept in SBUF, RoPE
fused into the projection pass with deferred pswap matmuls (no PE stalls),
software-pipelined attention inner loop (score k+1 issued before PV k so the
tensor engine never waits on the exp), and the ReduceScatter split into one
chunk per q-tile so it overlaps with subsequent compute.
"""

import os
import sys
import numpy as np

B, S, D = 2, 2048, 4096
H, KV, HD = 32, 8, 128
TP = 4
QH = H // TP          # 8 q heads per core
G = KV // TP          # 2 kv heads per core
P = 128
QT = 512              # q-tile for attention/projection (free dim)
NQT = S // QT         # 4
JT = 256              # projection j-tile (free dim)
NJT = S // JT         # 8
NDK = D // P          # 32 contraction chunks of 128
SCALE = float(HD) ** -0.5

LAST_EXEC_NS = None
LAST_TRACE_DIR = None


def _build():
    sys.path.insert(0, "/opt/trn_rl_repo")
    import concourse.bass as bass
    from concourse import bacc
    import concourse.mybir as mybir
    import concourse.tile as tile
    from contextlib import ExitStack

    F32 = mybir.dt.float32
    F32R = mybir.dt.float32r
    BF = mybir.dt.bfloat16
    Exp = mybir.ActivationFunctionType.Exp
    Copy = mybir.ActivationFunctionType.Copy
    MUL = mybir.AluOpType.mult
    ADD = mybir.AluOpType.add

    nc = bacc.Bacc(None, target_bir_lowering=False)
    xt_e = nc.dram_tensor("xt", [NJT, P, NDK, JT], BF, kind="ExternalInput")
    wq_e = nc.dram_tensor("wq", [QH, P, NDK, HD], BF, kind="ExternalInput")
    wk_e = nc.dram_tensor("wk", [P, NDK, G * HD], BF, kind="ExternalInput")
    wv_e = nc.dram_tensor("wv", [P, NDK, G * HD], BF, kind="ExternalInput")
    wo_e = nc.dram_tensor("wo", [P, QH, D], BF, kind="ExternalInput")
    cost_e = nc.dram_tensor("cost", [P, S], BF, kind="ExternalInput")
    sint_e = nc.dram_tensor("sint", [P, S], BF, kind="ExternalInput")
    mbig_e = nc.dram_tensor("mbig", [P, 1024], BF, kind="ExternalInput")
    onec_e = nc.dram_tensor("onec", [P, 1], BF, kind="ExternalInput")
    oner_e = nc.dram_tensor("oner", [1, P], F32, kind="ExternalInput")
    pswap_e = nc.dram_tensor("pswap", [P, P], BF, kind="ExternalInput")
    NO_CC = os.environ.get("KERNEL_NO_CC", "0") == "1"
    out_shape = [S, D] if NO_CC else [NQT, P, D]
    out_e = nc.dram_tensor("out", out_shape, BF, kind="ExternalOutput")

    with ExitStack() as top:
        top.enter_context(nc.allow_low_precision(reason="bf16 attention"))
        tc = top.enter_context(tile.TileContext(nc))
        const = top.enter_context(tc.tile_pool(name="const", bufs=1))
        mbig = const.tile([P, 1024], BF)
        onec = const.tile([P, 1], BF)
        oner = const.tile([1, P], F32)
        cosT = const.tile([P, S], BF)
        sinT = const.tile([P, S], BF)
        pswap = const.tile([P, P], BF)

        pers = top.enter_context(tc.tile_pool(name="pers", bufs=1))
        qT = pers.tile([P, QH, S], BF, name="qT")
        kT = pers.tile([P, G, S], BF, name="kT")
        vsb = pers.tile([P, S // P, G * HD], BF, name="vsb")

        dram = top.enter_context(tc.tile_pool(name="dram", bufs=1,
                                              space="DRAM"))
        partall = dram.tile([S, D], BF, name="partall")
        ccout = None if NO_CC else dram.tile([NQT, P, D], BF, name="ccout")

        # ---------------- phase A: QKV projections + RoPE ----------------
        with tc.tile_pool(name="wqkv", bufs=1) as w_pool, \
             tc.tile_pool(name="xtp", bufs=2) as xt_pool, \
             tc.tile_pool(name="rtmp", bufs=3) as rtmp_pool, \
             tc.tile_pool(name="psA", bufs=4, space="PSUM") as psA:
            # x-tile j=0 first so the first projection chain starts early
            xt_next = xt_pool.tile([P, NDK, JT], BF, name="xt_t")
            nc.sync.dma_start(xt_next[:], xt_e[0])
            wq_sb = []
            for h in range(QH):
                w = w_pool.tile([P, NDK, HD], BF, name=f"wq{h}")
                nc.sync.dma_start(w[:], wq_e[h])
                wq_sb.append(w)
            wk_sb = w_pool.tile([P, NDK, G * HD], BF, name="wk_sb")
            nc.sync.dma_start(wk_sb[:], wk_e[:])
            wv_sb = w_pool.tile([P, NDK, G * HD], BF, name="wv_sb")
            nc.sync.dma_start(wv_sb[:], wv_e[:])
            # consts are not needed until the first RoPE / attention, so
            # they queue behind the latency-critical xt/weight loads
            nc.sync.dma_start(cosT[:], cost_e[:])
            nc.sync.dma_start(sinT[:], sint_e[:])
            nc.sync.dma_start(pswap[:], pswap_e[:])
            nc.sync.dma_start(onec[:], onec_e[:])
            nc.sync.dma_start(oner[:], oner_e[:])
            nc.sync.dma_start(mbig[:], mbig_e[:])

            for j in range(NJT):
                sl = slice(j * JT, (j + 1) * JT)
                xt_t = xt_next
                if j + 1 < NJT:
                    xt_next = xt_pool.tile([P, NDK, JT], BF, name="xt_t")
                    nc.sync.dma_start(xt_next[:], xt_e[j + 1])
                ropes = []
                for h in range(QH):
                    ps = psA.tile([P, JT], F32, tag="qk")
                    for dk in range(NDK):
                        nc.tensor.matmul(
                            ps[:], wq_sb[h][:, dk, :], xt_t[:, dk, :],
                            start=(dk == 0), stop=(dk == NDK - 1))
                    tmp = rtmp_pool.tile([P, JT], BF, tag="rt", bufs=12)
                    nc.scalar.activation(tmp[:], ps[:], Copy)
                    ropes.append((tmp, qT[:, h, sl]))
                for g in range(G):
                    ps = psA.tile([P, JT], F32, tag="qk")
                    for dk in range(NDK):
                        nc.tensor.matmul(
                            ps[:], wk_sb[:, dk, g * HD:(g + 1) * HD],
                            xt_t[:, dk, :],
                            start=(dk == 0), stop=(dk == NDK - 1))
                    tmp = rtmp_pool.tile([P, JT], BF, tag="rt", bufs=12)
                    nc.scalar.activation(tmp[:], ps[:], Copy)
                    ropes.append((tmp, kT[:, g, sl]))
                for sub in range(2):
                    psv = psA.tile([P, G * HD], F32, tag="v", bufs=2)
                    for dk in range(NDK):
                        nc.tensor.matmul(
                            psv[:], xt_t[:, dk, sub * P:(sub + 1) * P],
                            wv_sb[:, dk, :],
                            start=(dk == 0), stop=(dk == NDK - 1))
                    nc.scalar.activation(vsb[:, j * 2 + sub, :], psv[:], Copy)
                # deferred RoPE: pswap matmuls run back-to-back on PE, the
                # vector work overlaps the next j-tile's projection chains
                for tmp, dst in ropes:
                    ps2 = psA.tile([P, JT], F32, tag="rope", bufs=2)
                    nc.tensor.matmul(ps2[:], pswap[:], tmp[:],
                                     start=True, stop=True)
                    d1 = rtmp_pool.tile([P, JT], BF, tag="d1", bufs=2)
                    nc.vector.tensor_tensor(d1[:], tmp[:], cosT[:, sl], MUL)
                    d2 = rtmp_pool.tile([P, JT], BF, tag="d2", bufs=2)
                    nc.vector.tensor_tensor(d2[:], ps2[:], sinT[:, sl], MUL)
                    nc.vector.tensor_tensor(dst, d1[:], d2[:], ADD)

        # ---------------- phase B: attention + output projection ----------------
        with tc.tile_pool(name="wop", bufs=1) as wo_pool, \
             tc.tile_pool(name="attn", bufs=2) as attn_pool, \
             tc.tile_pool(name="probs", bufs=3) as probs_pool, \
             tc.tile_pool(name="osbp", bufs=8) as osb_pool, \
             tc.tile_pool(name="rp", bufs=2) as rp_pool, \
             tc.tile_pool(name="psB", bufs=1, space="PSUM") as psB:
            wo_sb = wo_pool.tile([P, QH, D], BF, name="wo_sb")
            nc.sync.dma_start(wo_sb[:], wo_e[:])

            for t in range(NQT):
                q0 = t * QT
                sl = slice(q0, q0 + QT)
                nk = 4 * (t + 1)
                attn_t = attn_pool.tile([P, QH, QT], BF, tag="attn")

                # software-pipelined attention: score (h,ki) is issued one
                # step ahead of pv/sum of the previous pair, so the PE never
                # waits on the scalar-engine exp
                pairs = [(h, ki) for h in range(QH) for ki in range(nk)]
                state = {}

                def score(h, ki):
                    # diagonal blocks only cover q >= k: narrow the free dim
                    g = h // 4
                    k0 = ki * P
                    off = k0 - q0
                    o = max(off, 0)      # q-offset within the tile
                    w = QT - o
                    ps_s = psB.tile([P, QT], F32, tag="s", bufs=3)
                    nc.tensor.matmul(
                        ps_s[:, :w], kT[:, g, k0:k0 + P],
                        qT[:, h, q0 + o:q0 + QT],
                        start=True, stop=True)
                    pr = probs_pool.tile([P, QT], BF, tag="pr", bufs=4)
                    nc.scalar.activation(pr[:, :w], ps_s[:, :w], Exp,
                                         scale=SCALE)
                    if off >= 0:
                        nc.vector.tensor_tensor(
                            pr[:, :w], pr[:, :w],
                            mbig[:, 512:1024 - off], MUL)
                    return pr, o, w

                def post(h, ki, pr, o, w):
                    g = h // 4
                    if ki == 0:
                        state[h] = (
                            psB.tile([P, QT], F32, tag="pv", bufs=2,
                                     name="pv_ps"),
                            psB.tile([1, QT], F32, tag="sm", bufs=1,
                                     name="sm_ps"))
                    pv, sm = state[h]
                    nc.tensor.matmul(
                        pv[:, o:], vsb[:, ki, g * HD:(g + 1) * HD],
                        pr[:, :w],
                        start=(ki == 0), stop=(ki == nk - 1))
                    nc.tensor.matmul(sm[:, o:], onec[:], pr[:, :w],
                                     start=(ki == 0), stop=(ki == nk - 1))
                    if ki == nk - 1:
                        recip = rp_pool.tile([1, QT], F32, tag="recip")
                        nc.vector.reciprocal_approx_fast(recip[:], sm[:])
                        state[h] = (pv, recip)

                def finish(h):
                    pv, recip = state.pop(h)
                    ps_b = psB.tile([P, QT], F32, tag="s", bufs=3)
                    nc.tensor.matmul(ps_b[:], oner[:], recip[:],
                                     start=True, stop=True)
                    dst = attn_t[:, h, :]
                    nc.scalar.activation(dst, pv[:], Copy)
                    nc.vector.tensor_tensor(dst, dst, ps_b[:], MUL)

                pend = []
                done_h = -1
                for h, ki in pairs:
                    pr, o, w = score(h, ki)
                    if len(pend) == 2:
                        pp = pend.pop(0)
                        post(*pp)
                        if pp[1] == nk - 1:
                            done_h = pp[0]
                    if ki == 3 and done_h >= 0 and done_h == h - 1:
                        finish(done_h)
                        done_h = -1
                    pend.append((h, ki, pr, o, w))
                for pp in pend:
                    post(*pp)
                finish(QH - 1)

                # output projection for this q-tile. The ReduceScatter for
                # tiles 0-2 is emitted whole right after the projection so
                # its ring DMA overlaps the (write-free) attention of the
                # next tile; the last tile is split in half so most of it
                # overlaps the tail of the projection.
                groups = [[0, 1, 2, 3], [4, 5, 6, 7]]
                last = t == NQT - 1
                for si in range(4):
                    r0 = q0 + si * P
                    # tiles 0-2: full-width staging, one 8KB-line write per
                    # strip (fast rings). Last tile: narrow writes so each
                    # strip's ReduceScatter quarter can start immediately.
                    if not last:
                        osb = osb_pool.tile([P, D], BF, tag="ob", bufs=3)
                    for n in range(8):
                        n0 = n * QT
                        ps_o = psB.tile([P, QT], F32, tag="proj", bufs=2)
                        for hh in range(QH):
                            nc.tensor.matmul(
                                ps_o[:], attn_t[:, hh, si * P:(si + 1) * P],
                                wo_sb[:, hh, n0:n0 + QT],
                                start=(hh == 0), stop=(hh == QH - 1))
                        if last:
                            osb_n = osb_pool.tile([P, QT], BF, tag="obn",
                                                  bufs=6)
                            nc.scalar.activation(osb_n[:], ps_o[:], Copy)
                            nc.sync.dma_start(
                                partall[r0:r0 + P, n0:n0 + QT], osb_n[:])
                        else:
                            nc.scalar.activation(osb[:, n0:n0 + QT],
                                                 ps_o[:], Copy)
                    if not last:
                        nc.sync.dma_start(partall[r0:r0 + P, :], osb[:])
                    if not NO_CC and t == NQT - 1:
                        s32 = P // TP
                        nc.gpsimd.collective_compute(
                            "ReduceScatter", ADD, replica_groups=groups,
                            ins=[partall[r0:r0 + P, :].opt()],
                            outs=[ccout[t, si * s32:(si + 1) * s32, :].opt()])
                        nc.sync.dma_start(
                            out_e[t, si * s32:(si + 1) * s32, :],
                            ccout[t, si * s32:(si + 1) * s32, :])
                if NO_CC:
                    nc.sync.dma_start(
                        out_e[q0:q0 + QT, :], partall[q0:q0 + QT, :])
                elif t < NQT - 1:
                    nc.gpsimd.collective_compute(
                        "ReduceScatter", ADD, replica_groups=groups,
                        ins=[partall[q0:q0 + QT, :].opt()],
                        outs=[ccout[t].opt()])
                    nc.sync.dma_start(out_e[t], ccout[t])

    nc.compile()
    return nc


def _prep_in_maps(x, wq, wk, wv, wo, cos, sin):
    import ml_dtypes
    BF = ml_dtypes.bfloat16

    cosT = np.empty((HD, S), np.float32)
    sinT = np.empty((HD, S), np.float32)
    cosT[0::2] = cos.T
    cosT[1::2] = cos.T
    sinT[0::2] = -sin.T
    sinT[1::2] = sin.T
    cosT = cosT.astype(BF)
    sinT = sinT.astype(BF)
    mbig = (np.arange(1024)[None, :] >= (np.arange(P)[:, None] + 512)
            ).astype(BF)
    onec = np.ones((P, 1), BF)
    oner = np.ones((1, P), np.float32)
    pswap = np.zeros((P, P), np.float32)
    idx = np.arange(P)
    pswap[idx, idx ^ 1] = 1.0
    pswap = pswap.astype(BF)

    # x[b] pretiled: xt[j, p, dk*JT+s'] = x[b, j*JT+s', dk*P+p]
    xts = []
    for b in range(B):
        xr = x[b].reshape(NJT, JT, NDK, P)
        xts.append(np.ascontiguousarray(
            xr.transpose(0, 3, 2, 1)).astype(BF))

    in_maps = []
    for c in range(8):
        b, rk = c // TP, c % TP
        # wq head-major: wqp[h, p, dk*HD+m] = wq[dk*P+p, (rk*QH+h)*HD+m]
        wq_s = wq[:, rk * QH * HD:(rk + 1) * QH * HD]
        wqp = np.ascontiguousarray(
            wq_s.reshape(NDK, P, QH, HD).transpose(2, 1, 0, 3)).astype(BF)
        wk_s = wk[:, rk * G * HD:(rk + 1) * G * HD]
        wv_s = wv[:, rk * G * HD:(rk + 1) * G * HD]
        wo_s = wo[rk * QH * HD:(rk + 1) * QH * HD, :]
        in_maps.append({
            "xt": xts[b],
            "wq": wqp,
            "wk": np.ascontiguousarray(
                wk_s.reshape(NDK, P, G * HD).transpose(1, 0, 2)).astype(BF),
            "wv": np.ascontiguousarray(
                wv_s.reshape(NDK, P, G * HD).transpose(1, 0, 2)).astype(BF),
            "wo": np.ascontiguousarray(
                wo_s.reshape(QH, P, D).transpose(1, 0, 2)).astype(BF),
            "cost": cosT, "sint": sinT, "mbig": mbig,
            "onec": onec, "oner": oner, "pswap": pswap,
        })
    return in_maps


def _ensure_profile_hook():
    """Register the axon NTFF profile hook if the antenv plumbing is absent."""
    try:
        import antenv.axon_hooks  # noqa: F401
        return
    except ImportError:
        pass
    import types
    mod = types.ModuleType("antenv.axon_hooks")
    mod._HOOK = None

    def set_axon_ntff_profile_hook(hook):
        mod._HOOK = hook

    def get_axon_ntff_profile_hook():
        return mod._HOOK

    mod.set_axon_ntff_profile_hook = set_axon_ntff_profile_hook
    mod.get_axon_ntff_profile_hook = get_axon_ntff_profile_hook
    sys.modules["antenv.axon_hooks"] = mod
    try:
        from trn_agent_boot.trn_boot import _ntff_profile_via_ctypes
        hook = _ntff_profile_via_ctypes("/opt/axon/libaxon_pjrt.so")
        if hook is not None:
            mod._HOOK = hook
    except Exception:
        pass


def kernel(x, wq, wk, wv, wo, cos, sin, mask=None, positions=None, **_):
    global LAST_EXEC_NS, LAST_TRACE_DIR
    x = np.asarray(x, np.float32)
    wq = np.asarray(wq, np.float32)
    wk = np.asarray(wk, np.float32)
    wv = np.asarray(wv, np.float32)
    wo = np.asarray(wo, np.float32)
    cos = np.asarray(cos, np.float32)
    sin = np.asarray(sin, np.float32)

    sys.path.insert(0, "/opt/trn_rl_repo")
    from concourse.bass_utils import run_bass_kernel_spmd

    nc = _build()
    in_maps = _prep_in_maps(x, wq, wk, wv, wo, cos, sin)
    want_trace = os.environ.get("BASS_TRACE", "1") != "0"
    res = None
    if want_trace:
        try:
            _ensure_profile_hook()
            res = run_bass_kernel_spmd(nc, in_maps, list(range(8)), trace=True)
        except Exception:
            res = None
    if res is None:
        res = run_bass_kernel_spmd(nc, in_maps, list(range(8)), trace=False)
    LAST_EXEC_NS = res.exec_time_ns
    if LAST_EXEC_NS is None and os.environ.get("BASS_WALLTIME", "1") == "1":
        import time as _time
        t0 = _time.perf_counter()
        res = run_bass_kernel_spmd(nc, in_maps, list(range(8)), trace=False)
        LAST_EXEC_NS = int((_time.perf_counter() - t0) * 1e9)
    try:
        LAST_TRACE_DIR = getattr(res, "profile_json", None)
    except Exception:
        LAST_TRACE_DIR = None

    out = np.empty((B, S, D), np.float32)
    if os.environ.get("KERNEL_NO_CC", "0") == "1":
        for b in range(B):
            out[b] = sum(
                res.results[b * TP + rk]["out"].astype(np.float32)
                for rk in range(TP))
    else:
        for c in range(8):
            b, rk = c // TP, c % TP
            o = res.results[c]["out"].astype(np.float32)
            for t in range(NQT - 1):
                r0 = t * QT + rk * P
                out[b, r0:r0 + P, :] = o[t]
            # last tile was reduce-scattered per 128-row strip
            t = NQT - 1
            s32 = P // TP
            for si in range(4):
                r0 = t * QT + si * P + rk * s32
                out[b, r0:r0 + s32, :] = o[t, si * s32:(si + 1) * s32]
    return out
